# revision 36
# baseline (speedup 1.0000x reference)
"""Distributed CLIP loss on 8 TRN2 NeuronCores.

Contract: kernel(**inputs) takes the FULL inputs
  image_features (8192, 1024) f32, text_features (8192, 1024) f32,
  logit_scale () f32
and returns the FULL output: scalar f32 loss.

Strategy (data parallel over batch):
  - shard image rows 8 ways (1024 rows/core); every core gets all text rows
  - core computes L^T tile-block = logit_scale * text_tile @ I_c^T via
    TensorE (bf16, fp32 PSUM accumulation)
  - E = exp(L^T - C) on ScalarE; its accum_out gives the text-direction
    (t2i) partial column sums for free; VectorE accumulates A += E for the
    image-direction (i2t) row sums
  - diagonal logits computed as elementwise product + reduce (exact rows
    of text matching this core's image rows are passed as an extra input)
  - one 33 KB AllReduce combines the t2i partial sums + per-core scalars;
    every core finishes the identical final scalar
Host side only reshapes/casts inputs (layout prep) and reads back core 0's
scalar.
"""

import numpy as np
import ml_dtypes

import concourse.bass as bass
import concourse.tile as tile
from concourse import bacc, mybir
from concourse.bass_utils import run_bass_kernel_spmd

N = 8192
D = 1024
CORES = 8
LOCAL = N // CORES          # image rows per core
KC = D // 128               # contraction chunks of 128
TT = N // 128               # text tiles of 128 rows
C_SHIFT = 25.0              # exp shift: logits for this data are within ~[-20, 20]
ARSPLIT = 48                # t2i columns all-reduced early (overlapped)

# fp8 e4m3 + DoubleRow doubles TensorE throughput; measured end-to-end
# rel err ~6e-4 on this problem (vs ~1.6e-5 for bf16).
USE_FP8 = True
PRE_I = 8.0                 # image prescale (power of two, exact)
PRE_T = 32.0                # text prescale
INV_PRE = 1.0 / (PRE_I * PRE_T)

BF16 = mybir.dt.bfloat16
FP8 = mybir.dt.float8e4
F32 = mybir.dt.float32
AF = mybir.ActivationFunctionType
ALU = mybir.AluOpType

_CACHE = {}


class _Bacc(bacc.Bacc):
    def insert_act_table_loads(self):
        """Steer Exp/Ln/Copy to the one table set containing all three, so the
        kernel pays a single ACT table load instead of four exp<->ln swaps
        (the swap otherwise lands mid-stream and stalls ScalarE ~1.3us)."""
        from concourse.hw_specs import get_activation_tables

        has_activation = any(
            isinstance(i, mybir.InstActivation)
            for b in self.main_func.blocks
            for i in b.instructions
        )
        if not has_activation:
            return
        ours = {AF.Exp, AF.Ln, AF.Copy}
        tables = []
        for name, fns in get_activation_tables(self.m.arch).items():
            if name != "natural_log_exp_and_others":
                fns = fns - ours
            tables.append((name, fns))
        bacc._bass_rust.insert_act_table_loads(self, tables)


def _build_bass():
    from contextlib import ExitStack

    nc = _Bacc(None, num_devices=CORES)
    in_dt = FP8 if USE_FP8 else BF16
    # matmul output is prescaled by PRE_I*PRE_T in fp8 mode
    unscale = INV_PRE if USE_FP8 else 1.0

    # it: scaled image shard, transposed:   it[p, kc, i] = s*PRE_I * I_c[i, kc*128+p]
    # tt: all text rows, tiled+transposed:  tt[t, p, kc, c] = PRE_T * T[t*128+c, kc*128+p]
    # td: text shard matching local images: td[p, kc, i] = PRE_T * T_c[i, kc*128+p]
    it_d = nc.declare_dram_parameter("it", [128, KC, LOCAL], in_dt, isOutput=False)
    tt_d = nc.declare_dram_parameter("tt", [TT, 128, KC, 128], in_dt, isOutput=False)
    td_d = nc.declare_dram_parameter("td", [128, KC, LOCAL], in_dt, isOutput=False)
    out_d = nc.declare_dram_parameter("out", [1, 1], F32, isOutput=True)

    with tile.TileContext(nc) as tc, ExitStack() as ctx:
        singles = ctx.enter_context(tc.tile_pool(name="singles", bufs=1))
        tpool = ctx.enter_context(tc.tile_pool(name="tpool", bufs=4))
        epool = ctx.enter_context(tc.tile_pool(name="epool", bufs=4))
        ppool = ctx.enter_context(tc.tile_pool(name="ppool", bufs=2, space="PSUM"))
        tailp = ctx.enter_context(tc.tile_pool(name="tailp", bufs=2, space="PSUM"))
        drp = ctx.enter_context(tc.tile_pool(name="drp", bufs=1, space="DRAM"))

        # Fine-grained IT chunks across both DMA families so the first matmul
        # group can start as soon as its slices land; h=0 halves arrive first.
        IT = singles.tile([128, KC, LOCAL], in_dt)
        nc.sync.dma_start(out=IT, in_=it_d[:])

        A = singles.tile([128, LOCAL], F32)        # i2t partial sums by text-partition
        nc.vector.memset(A, 0.0)
        ones = singles.tile([128, 1], F32)
        nc.vector.memset(ones, 1.0)
        nshift = singles.tile([128, 1], F32)
        nc.vector.memset(nshift, -C_SHIFT)
        # Warm the ACT exp/ln table set during the input DMA so the first real
        # exp doesn't stall 2.7us on PSEUDO_LOAD_ACT_FUNC_SET.
        actwarm = singles.tile([1, 2], F32)
        nc.scalar.activation(actwarm[:, 0:1], ones[0:1, :], AF.Exp)
        nc.scalar.activation(actwarm[:, 1:2], ones[0:1, :], AF.Ln)
        # t2i partial sums land in cols 0..63; col 64 = i2t ln-sum scalar,
        # col 65 = diag total (scalars on partition 0 only).
        arin = singles.tile([128, 66], F32)
        colacc = arin[:, 0:TT]
        nc.vector.memset(arin[:, TT:TT + 2], 0.0)

        TD = singles.tile([128, KC, LOCAL], in_dt)
        dprod = singles.tile([128, KC, LOCAL], F32)
        dacc8 = singles.tile([128, KC], F32)

        arsum = singles.tile([128, 66], F32)
        cc_in1 = drp.tile([128, ARSPLIT], F32)
        cc_out1 = drp.tile([128, ARSPLIT], F32, addr_space="Shared")
        cc_in2 = drp.tile([128, 66 - ARSPLIT], F32)
        cc_out2 = drp.tile([128, 66 - ARSPLIT], F32, addr_space="Shared")

        # ---- main loop: 64 text tiles x (matmuls + exp + accumulate) ----
        for t in range(TT):
            ttile = tpool.tile([128, KC, 128], in_dt)
            nc.sync.dma_start(out=ttile, in_=tt_d[t])
            psum = ppool.tile([128, LOCAL], F32)
            for h in range(2):
                if USE_FP8:
                    # DoubleRow: contraction over (partition, kc-pair), 2 fp8
                    # weights per PE cell -> half the matmuls of bf16.
                    for kp in range(KC // 2):
                        nc.tensor.matmul(
                            psum[:, h * 512:(h + 1) * 512],
                            ttile[:, 2 * kp:2 * kp + 2, :],
                            IT[:, 2 * kp:2 * kp + 2, h * 512:(h + 1) * 512],
                            start=(kp == 0),
                            stop=(kp == KC // 2 - 1),
                            perf_mode=mybir.MatmulPerfMode.DoubleRow,
                        )
                else:
                    for kc in range(KC):
                        nc.tensor.matmul(
                            psum[:, h * 512:(h + 1) * 512],
                            ttile[:, kc, :],
                            IT[:, kc, h * 512:(h + 1) * 512],
                            start=(kc == 0),
                            stop=(kc == KC - 1),
                        )
            E = epool.tile([128, LOCAL], F32)
            nc.scalar.activation(
                E, psum, AF.Exp, bias=nshift, scale=unscale,
                accum_out=colacc[:, t:t + 1],
            )
            nc.vector.tensor_add(A, A, E)
            # Interleave the diagonal (sum_i s*<I_c[i], T_c[i]>) into DVE slack
            # mid-loop: TD shard DMA after a few tiles, one kc-chunk of the
            # elementwise product every few text tiles.
            if t == 1:
                nc.sync.dma_start(out=TD, in_=td_d[:])
            if t >= 4 and t % 4 == 0 and t // 4 <= KC:
                kc = t // 4 - 1
                nc.vector.scalar_tensor_tensor(
                    out=dprod[:, kc, :], in0=IT[:, kc, :], scalar=unscale,
                    in1=TD[:, kc, :],
                    op0=ALU.mult, op1=ALU.mult,
                    accum_out=dacc8[:, kc:kc + 1],
                )
            if t == ARSPLIT - 1:
                # First 48 t2i partial columns are final -> AllReduce them
                # under the remaining ~16 tiles of compute (also absorbs the
                # collective's first-call overhead off the critical path).
                nc.sync.dma_start(out=cc_in1, in_=arin[:, 0:ARSPLIT])
                nc.gpsimd.collective_compute(
                    "AllReduce",
                    ALU.add,
                    replica_groups=[list(range(CORES))],
                    ins=[cc_in1[:]],
                    outs=[cc_out1[:]],
                )
                nc.sync.dma_start(out=arsum[:, 0:ARSPLIT], in_=cc_out1)

        # ---- diag chunk totals -> one per-partition scalar ----
        dacc = singles.tile([128, 1], F32)
        nc.vector.tensor_reduce(
            out=dacc, in_=dacc8, op=ALU.add, axis=mybir.AxisListType.X,
        )

        # ---- i2t row sums: partition-reduce A via ones-matmul, then ln ----
        psum_r = tailp.tile([1, LOCAL], F32, tag="tail")
        for h in range(2):
            nc.tensor.matmul(
                psum_r[:, h * 512:(h + 1) * 512], ones, A[:, h * 512:(h + 1) * 512],
                start=True, stop=True,
            )
        lse_r = singles.tile([1, LOCAL], F32)
        s_row = singles.tile([1, 1], F32)
        nc.scalar.activation(lse_r, psum_r, AF.Ln, accum_out=s_row)

        # ---- diag total: partition-reduce dacc ----
        psum_d = tailp.tile([1, LOCAL], F32, tag="tail")
        nc.tensor.matmul(psum_d[:, 0:1], ones, dacc, start=True, stop=True)

        # ---- AllReduce #2: remaining t2i columns + the two scalars ----
        nc.vector.tensor_copy(arin[0:1, TT:TT + 1], s_row)
        nc.vector.tensor_copy(arin[0:1, TT + 1:TT + 2], psum_d[0:1, 0:1])

        nc.sync.dma_start(out=cc_in2, in_=arin[:, ARSPLIT:66])
        nc.gpsimd.collective_compute(
            "AllReduce",
            ALU.add,
            replica_groups=[list(range(CORES))],
            ins=[cc_in2[:]],
            outs=[cc_out2[:]],
        )
        nc.sync.dma_start(out=arsum[:, ARSPLIT:66], in_=cc_out2)

        # ---- t2i logsumexp over the reduced column sums ----
        lse_c = singles.tile([128, TT], F32)
        cl = singles.tile([128, 1], F32)
        nc.scalar.activation(lse_c, arsum[:, 0:TT], AF.Ln, accum_out=cl)
        psum_s = tailp.tile([1, LOCAL], F32, tag="tail")
        nc.tensor.matmul(psum_s[:, 0:1], ones, cl, start=True, stop=True)

        # ---- loss = (S_col + S_row - 2*diag_tot)/(2N) + C ----
        f0 = singles.tile([1, 1], F32)
        nc.vector.tensor_add(f0, psum_s[0:1, 0:1], arsum[0:1, TT:TT + 1])
        f1 = singles.tile([1, 1], F32)
        nc.vector.scalar_tensor_tensor(
            out=f1, in0=arsum[0:1, TT + 1:TT + 2], scalar=-2.0, in1=f0,
            op0=ALU.mult, op1=ALU.add,
        )
        res = singles.tile([1, 1], F32)
        nc.scalar.activation(res, f1, AF.Copy, bias=C_SHIFT, scale=1.0 / (2.0 * N))
        nc.sync.dma_start(out=out_d[:], in_=res)

    nc.finalize()
    return nc


def _prep_inputs(image_features, text_features, logit_scale):
    s = float(np.asarray(logit_scale, dtype=np.float32))
    I = np.asarray(image_features, dtype=np.float32)
    T = np.asarray(text_features, dtype=np.float32)
    if USE_FP8:
        dt = ml_dtypes.float8_e4m3
        si, st = s * PRE_I, PRE_T
    else:
        dt = ml_dtypes.bfloat16
        si, st = s, 1.0

    # tt[t, p, kc, c] = st * T[t*128 + c, kc*128 + p]
    tt = np.ascontiguousarray(
        (T * st).reshape(TT, 128, KC, 128).transpose(0, 3, 2, 1)
    ).astype(dt)

    in_maps = []
    for c in range(CORES):
        Ic = I[c * LOCAL:(c + 1) * LOCAL] * si
        it = np.ascontiguousarray(
            Ic.reshape(LOCAL, KC, 128).transpose(2, 1, 0)
        ).astype(dt)
        Tc = T[c * LOCAL:(c + 1) * LOCAL] * st
        td = np.ascontiguousarray(
            Tc.reshape(LOCAL, KC, 128).transpose(2, 1, 0)
        ).astype(dt)
        in_maps.append({"it": it, "tt": tt, "td": td})
    return in_maps


def _get_nc():
    if "nc" not in _CACHE:
        _CACHE["nc"] = _build_bass()
    return _CACHE["nc"]


def kernel(image_features, text_features, logit_scale, _trace=False):
    nc = _get_nc()
    in_maps = _prep_inputs(image_features, text_features, logit_scale)
    out = run_bass_kernel_spmd(nc, in_maps, list(range(CORES)), trace=_trace)
    loss = np.float32(out.results[0]["out"][0, 0])
    if _trace:
        return loss, out
    return loss


# revision 38
# speedup vs baseline: 1.0550x; 1.0550x over previous
"""Distributed CLIP loss on 8 TRN2 NeuronCores.

Contract: kernel(**inputs) takes the FULL inputs
  image_features (8192, 1024) f32, text_features (8192, 1024) f32,
  logit_scale () f32
and returns the FULL output: scalar f32 loss.

Strategy (data parallel over batch):
  - shard image rows 8 ways (1024 rows/core); every core gets all text rows
  - core computes L^T tile-block = logit_scale * text_tile @ I_c^T via
    TensorE (bf16, fp32 PSUM accumulation)
  - E = exp(L^T - C) on ScalarE; its accum_out gives the text-direction
    (t2i) partial column sums for free; VectorE accumulates A += E for the
    image-direction (i2t) row sums
  - diagonal logits computed as elementwise product + reduce (exact rows
    of text matching this core's image rows are passed as an extra input)
  - one 33 KB AllReduce combines the t2i partial sums + per-core scalars;
    every core finishes the identical final scalar
Host side only reshapes/casts inputs (layout prep) and reads back core 0's
scalar.
"""

import numpy as np
import ml_dtypes

import concourse.bass as bass
import concourse.tile as tile
from concourse import bacc, mybir
from concourse.bass_utils import run_bass_kernel_spmd

N = 8192
D = 1024
CORES = 8
LOCAL = N // CORES          # image rows per core
KC = D // 128               # contraction chunks of 128
TT = N // 128               # text tiles of 128 rows
C_SHIFT = 25.0              # exp shift: logits for this data are within ~[-20, 20]
ARSPLIT = 32                # t2i columns all-reduced early (overlapped)

# fp8 e4m3 + DoubleRow doubles TensorE throughput; measured end-to-end
# rel err ~6e-4 on this problem (vs ~1.6e-5 for bf16).
USE_FP8 = True
PRE_I = 8.0                 # image prescale (power of two, exact)
PRE_T = 32.0                # text prescale
INV_PRE = 1.0 / (PRE_I * PRE_T)

BF16 = mybir.dt.bfloat16
FP8 = mybir.dt.float8e4
F32 = mybir.dt.float32
AF = mybir.ActivationFunctionType
ALU = mybir.AluOpType

_CACHE = {}


class _Bacc(bacc.Bacc):
    def insert_act_table_loads(self):
        """Steer Exp/Ln/Copy to the one table set containing all three, so the
        kernel pays a single ACT table load instead of four exp<->ln swaps
        (the swap otherwise lands mid-stream and stalls ScalarE ~1.3us)."""
        from concourse.hw_specs import get_activation_tables

        has_activation = any(
            isinstance(i, mybir.InstActivation)
            for b in self.main_func.blocks
            for i in b.instructions
        )
        if not has_activation:
            return
        ours = {AF.Exp, AF.Ln, AF.Copy}
        tables = []
        for name, fns in get_activation_tables(self.m.arch).items():
            if name != "natural_log_exp_and_others":
                fns = fns - ours
            tables.append((name, fns))
        bacc._bass_rust.insert_act_table_loads(self, tables)


def _build_bass():
    from contextlib import ExitStack

    nc = _Bacc(None, num_devices=CORES)
    in_dt = FP8 if USE_FP8 else BF16
    # matmul output is prescaled by PRE_I*PRE_T in fp8 mode
    unscale = INV_PRE if USE_FP8 else 1.0

    # it: scaled image shard, transposed:   it[p, kc, i] = s*PRE_I * I_c[i, kc*128+p]
    # tt: all text rows, tiled+transposed:  tt[t, p, kc, c] = PRE_T * T[t*128+c, kc*128+p]
    # td: text shard matching local images: td[p, kc, i] = PRE_T * T_c[i, kc*128+p]
    it_d = nc.declare_dram_parameter("it", [128, KC, LOCAL], in_dt, isOutput=False)
    tt_d = nc.declare_dram_parameter("tt", [TT, 128, KC, 128], in_dt, isOutput=False)
    td_d = nc.declare_dram_parameter("td", [128, KC, LOCAL], in_dt, isOutput=False)
    out_d = nc.declare_dram_parameter("out", [1, 1], F32, isOutput=True)

    with tile.TileContext(nc) as tc, ExitStack() as ctx:
        singles = ctx.enter_context(tc.tile_pool(name="singles", bufs=1))
        tpool = ctx.enter_context(tc.tile_pool(name="tpool", bufs=6))
        epool = ctx.enter_context(tc.tile_pool(name="epool", bufs=4))
        ppool = ctx.enter_context(tc.tile_pool(name="ppool", bufs=2, space="PSUM"))
        tailp = ctx.enter_context(tc.tile_pool(name="tailp", bufs=2, space="PSUM"))
        drp = ctx.enter_context(tc.tile_pool(name="drp", bufs=1, space="DRAM"))

        # Fine-grained IT chunks across both DMA families so the first matmul
        # group can start as soon as its slices land; h=0 halves arrive first.
        IT = singles.tile([128, KC, LOCAL], in_dt)
        nc.sync.dma_start(out=IT, in_=it_d[:])

        A = singles.tile([128, LOCAL], F32)        # i2t partial sums by text-partition
        nc.vector.memset(A, 0.0)
        ones = singles.tile([128, 1], F32)
        nc.vector.memset(ones, 1.0)
        nshift = singles.tile([128, 1], F32)
        nc.vector.memset(nshift, -C_SHIFT)
        # Warm the ACT exp/ln table set during the input DMA so the first real
        # exp doesn't stall 2.7us on PSEUDO_LOAD_ACT_FUNC_SET.
        actwarm = singles.tile([1, 2], F32)
        nc.scalar.activation(actwarm[:, 0:1], ones[0:1, :], AF.Exp)
        nc.scalar.activation(actwarm[:, 1:2], ones[0:1, :], AF.Ln)
        # t2i partial sums land in cols 0..63; col 64 = i2t ln-sum scalar,
        # col 65 = diag total (scalars on partition 0 only).
        arin = singles.tile([128, 66], F32)
        colacc = arin[:, 0:TT]
        nc.vector.memset(arin[:, TT:TT + 2], 0.0)

        TD = singles.tile([128, KC, LOCAL], in_dt)
        dprod = singles.tile([128, KC, LOCAL], F32)
        dacc8 = singles.tile([128, KC], F32)

        arsum = singles.tile([128, 66], F32)
        cc_in1 = drp.tile([128, ARSPLIT], F32)
        cc_out1 = drp.tile([128, ARSPLIT], F32, addr_space="Shared")
        cc_in2 = drp.tile([128, 66 - ARSPLIT], F32)
        cc_out2 = drp.tile([128, 66 - ARSPLIT], F32, addr_space="Shared")

        # ---- main loop: 64 text tiles x (matmuls + exp + accumulate) ----
        for t in range(TT):
            ttile = tpool.tile([128, KC, 128], in_dt)
            nc.sync.dma_start(out=ttile, in_=tt_d[t])
            psum = ppool.tile([128, LOCAL], F32)
            for h in range(2):
                if USE_FP8:
                    # DoubleRow: contraction over (partition, kc-pair), 2 fp8
                    # weights per PE cell -> half the matmuls of bf16.
                    for kp in range(KC // 2):
                        nc.tensor.matmul(
                            psum[:, h * 512:(h + 1) * 512],
                            ttile[:, 2 * kp:2 * kp + 2, :],
                            IT[:, 2 * kp:2 * kp + 2, h * 512:(h + 1) * 512],
                            start=(kp == 0),
                            stop=(kp == KC // 2 - 1),
                            perf_mode=mybir.MatmulPerfMode.DoubleRow,
                        )
                else:
                    for kc in range(KC):
                        nc.tensor.matmul(
                            psum[:, h * 512:(h + 1) * 512],
                            ttile[:, kc, :],
                            IT[:, kc, h * 512:(h + 1) * 512],
                            start=(kc == 0),
                            stop=(kc == KC - 1),
                        )
            E = epool.tile([128, LOCAL], F32)
            nc.scalar.activation(
                E, psum, AF.Exp, bias=nshift, scale=unscale,
                accum_out=colacc[:, t:t + 1],
            )
            nc.vector.tensor_add(A, A, E)
            # Interleave the diagonal (sum_i s*<I_c[i], T_c[i]>) into DVE slack
            # mid-loop: TD shard DMA after a few tiles, one kc-chunk of the
            # elementwise product every few text tiles.
            if t == 1:
                nc.sync.dma_start(out=TD, in_=td_d[:])
            if t >= 4 and t % 4 == 0 and t // 4 <= KC:
                kc = t // 4 - 1
                nc.vector.scalar_tensor_tensor(
                    out=dprod[:, kc, :], in0=IT[:, kc, :], scalar=unscale,
                    in1=TD[:, kc, :],
                    op0=ALU.mult, op1=ALU.mult,
                    accum_out=dacc8[:, kc:kc + 1],
                )
            if t == ARSPLIT - 1:
                # First 48 t2i partial columns are final -> AllReduce them
                # under the remaining ~16 tiles of compute (also absorbs the
                # collective's first-call overhead off the critical path).
                nc.sync.dma_start(out=cc_in1, in_=arin[:, 0:ARSPLIT])
                nc.gpsimd.collective_compute(
                    "AllReduce",
                    ALU.add,
                    replica_groups=[list(range(CORES))],
                    ins=[cc_in1[:]],
                    outs=[cc_out1[:]],
                )
                nc.sync.dma_start(out=arsum[:, 0:ARSPLIT], in_=cc_out1)

        # ---- diag chunk totals -> one per-partition scalar ----
        dacc = singles.tile([128, 1], F32)
        nc.vector.tensor_reduce(
            out=dacc, in_=dacc8, op=ALU.add, axis=mybir.AxisListType.X,
        )

        # ---- i2t row sums: partition-reduce A via ones-matmul, then ln ----
        psum_r = tailp.tile([1, LOCAL], F32, tag="tail")
        for h in range(2):
            nc.tensor.matmul(
                psum_r[:, h * 512:(h + 1) * 512], ones, A[:, h * 512:(h + 1) * 512],
                start=True, stop=True,
            )
        lse_r = singles.tile([1, LOCAL], F32)
        s_row = singles.tile([1, 1], F32)
        nc.scalar.activation(lse_r, psum_r, AF.Ln, accum_out=s_row)

        # ---- diag total: partition-reduce dacc ----
        psum_d = tailp.tile([1, LOCAL], F32, tag="tail")
        nc.tensor.matmul(psum_d[:, 0:1], ones, dacc, start=True, stop=True)

        # ---- AllReduce #2: remaining t2i columns + the two scalars ----
        nc.vector.tensor_copy(arin[0:1, TT:TT + 1], s_row)
        nc.vector.tensor_copy(arin[0:1, TT + 1:TT + 2], psum_d[0:1, 0:1])

        nc.sync.dma_start(out=cc_in2, in_=arin[:, ARSPLIT:66])
        nc.gpsimd.collective_compute(
            "AllReduce",
            ALU.add,
            replica_groups=[list(range(CORES))],
            ins=[cc_in2[:]],
            outs=[cc_out2[:]],
        )
        nc.sync.dma_start(out=arsum[:, ARSPLIT:66], in_=cc_out2)

        # ---- t2i logsumexp over the reduced column sums ----
        lse_c = singles.tile([128, TT], F32)
        cl = singles.tile([128, 1], F32)
        nc.scalar.activation(lse_c, arsum[:, 0:TT], AF.Ln, accum_out=cl)
        psum_s = tailp.tile([1, LOCAL], F32, tag="tail")
        nc.tensor.matmul(psum_s[:, 0:1], ones, cl, start=True, stop=True)

        # ---- loss = (S_col + S_row - 2*diag_tot)/(2N) + C ----
        f0 = singles.tile([1, 1], F32)
        nc.vector.tensor_add(f0, psum_s[0:1, 0:1], arsum[0:1, TT:TT + 1])
        f1 = singles.tile([1, 1], F32)
        nc.vector.scalar_tensor_tensor(
            out=f1, in0=arsum[0:1, TT + 1:TT + 2], scalar=-2.0, in1=f0,
            op0=ALU.mult, op1=ALU.add,
        )
        res = singles.tile([1, 1], F32)
        nc.scalar.activation(res, f1, AF.Copy, bias=C_SHIFT, scale=1.0 / (2.0 * N))
        nc.sync.dma_start(out=out_d[:], in_=res)

    nc.finalize()
    return nc


def _prep_inputs(image_features, text_features, logit_scale):
    s = float(np.asarray(logit_scale, dtype=np.float32))
    I = np.asarray(image_features, dtype=np.float32)
    T = np.asarray(text_features, dtype=np.float32)
    if USE_FP8:
        dt = ml_dtypes.float8_e4m3
        si, st = s * PRE_I, PRE_T
    else:
        dt = ml_dtypes.bfloat16
        si, st = s, 1.0

    # tt[t, p, kc, c] = st * T[t*128 + c, kc*128 + p]
    tt = np.ascontiguousarray(
        (T * st).reshape(TT, 128, KC, 128).transpose(0, 3, 2, 1)
    ).astype(dt)

    in_maps = []
    for c in range(CORES):
        Ic = I[c * LOCAL:(c + 1) * LOCAL] * si
        it = np.ascontiguousarray(
            Ic.reshape(LOCAL, KC, 128).transpose(2, 1, 0)
        ).astype(dt)
        Tc = T[c * LOCAL:(c + 1) * LOCAL] * st
        td = np.ascontiguousarray(
            Tc.reshape(LOCAL, KC, 128).transpose(2, 1, 0)
        ).astype(dt)
        in_maps.append({"it": it, "tt": tt, "td": td})
    return in_maps


def _get_nc():
    if "nc" not in _CACHE:
        _CACHE["nc"] = _build_bass()
    return _CACHE["nc"]


def kernel(image_features, text_features, logit_scale, _trace=False):
    nc = _get_nc()
    in_maps = _prep_inputs(image_features, text_features, logit_scale)
    out = run_bass_kernel_spmd(nc, in_maps, list(range(CORES)), trace=_trace)
    loss = np.float32(out.results[0]["out"][0, 0])
    if _trace:
        return loss, out
    return loss
